# revision 3
# baseline (speedup 1.0000x reference)
"""LoRA linear layer (out = x @ (W + s*A@B) + bias) on 8 Trainium2 NeuronCores.

Sharding: data-parallel over rows of x (M = 4*2048 = 8192 -> 1024 rows/core).
Each core computes its row-slice against the full weight matrix:
  - x slice is PE-transposed on device into [K, M] layout (fp32 -> fp32r)
  - main matmul: 32 K-tile fp32r matmuls accumulate into PSUM per
    [128m x 256n] output tile (stationary = xT tile, moving = W tile)
  - LoRA: xAT = (A^T x^T) computed once on device (rank 16), then a 33rd
    rank-16 matmul per output tile adds s*(xA)B into the same PSUM
  - bias added during the PSUM -> SBUF copy, result DMA'd out
"""
import numpy as np

import concourse.bass as bass
import concourse.tile as tile
from concourse import bacc, mybir
from concourse.bass_utils import run_bass_kernel_spmd
from concourse.masks import make_identity

P = 128
N_CORES = 8
BATCH, SEQ = 4, 2048
D_IN, D_OUT, RANK = 4096, 4096, 16
M_FULL = BATCH * SEQ          # 8192
M_C = M_FULL // N_CORES       # 1024 rows per core
KT = D_IN // P                # 32 k-tiles
MT = M_C // P                 # 8 m-tiles per core
W_N = 256                     # n-chunk width (moving free dim; >=256 keeps fp32r at full rate)
NCH = D_OUT // W_N            # 16 n-chunks
XQ = 1024                     # x load column-chunk (k) width
F32 = mybir.dt.float32
F32R = mybir.dt.float32r

_NC_CACHE = None


def _build_nc():
    nc = bacc.Bacc("TRN2", target_bir_lowering=False, debug=False,
                   num_devices=N_CORES)
    x_d = nc.dram_tensor("x", [M_C, D_IN], F32, kind="ExternalInput").ap()
    w_d = nc.dram_tensor("w", [D_IN, D_OUT], F32, kind="ExternalInput").ap()
    bias_d = nc.dram_tensor("bias", [D_OUT], F32, kind="ExternalInput").ap()
    a_d = nc.dram_tensor("lora_a", [D_IN, RANK], F32, kind="ExternalInput").ap()
    b_d = nc.dram_tensor("lora_b", [RANK, D_OUT], F32, kind="ExternalInput").ap()
    out_d = nc.dram_tensor("out", [M_C, D_OUT], F32, kind="ExternalOutput").ap()

    with tile.TileContext(nc) as tc:
        with (
            tc.tile_pool(name="singles", bufs=1) as singles,
            tc.tile_pool(name="xin", bufs=2) as xin_pool,
            tc.tile_pool(name="wts", bufs=36) as w_pool,
            tc.tile_pool(name="bt", bufs=3) as b_pool,
            tc.tile_pool(name="biast", bufs=3) as bias_pool,
            tc.tile_pool(name="outs", bufs=4) as out_pool,
            tc.tile_pool(name="psum", bufs=8, space="PSUM") as psum_pool,
        ):
            ident = singles.tile([P, P], F32)
            make_identity(nc, ident)

            # lora_A striped k-on-partitions: A_sb[p, kt, r] = A[kt*128+p, r]
            a_sb = singles.tile([P, KT, RANK], F32R)
            nc.sync.dma_start(
                out=a_sb,
                in_=a_d.bitcast(F32R).rearrange("(kt p) r -> p kt r", p=P),
            )

            # x slice transposed to [k, m] layout, rounded to fp32r
            xT = singles.tile([P, KT, M_C], F32R)
            for mt in range(MT):
                for kq in range(D_IN // XQ):
                    xin = xin_pool.tile([P, XQ], F32)
                    nc.sync.dma_start(
                        out=xin,
                        in_=x_d[mt * P:(mt + 1) * P, kq * XQ:(kq + 1) * XQ],
                    )
                    for ks in range(XQ // P):
                        kt = kq * (XQ // P) + ks
                        pt = psum_pool.tile([P, P], F32, tag="ps")
                        nc.tensor.transpose(
                            pt, xin[:, ks * P:(ks + 1) * P], ident)
                        nc.vector.tensor_copy(
                            out=xT[:, kt, mt * P:(mt + 1) * P], in_=pt)

            # xAT[r, m] = sum_k A[k, r] * x[m, k]   (psum accumulated, rank 16)
            xat = singles.tile([RANK, M_C], F32R)
            for mc in range(M_C // 512):
                xp = psum_pool.tile([RANK, 512], F32, tag="ps")
                for kt in range(KT):
                    nc.tensor.matmul(
                        xp,
                        a_sb[:, kt, :],
                        xT[:, kt, mc * 512:(mc + 1) * 512],
                        start=(kt == 0),
                        stop=(kt == KT - 1),
                    )
                nc.vector.tensor_copy(
                    out=xat[:, mc * 512:(mc + 1) * 512], in_=xp)

            # main loop over output column chunks
            for nci in range(NCH):
                nsl = slice(nci * W_N, (nci + 1) * W_N)
                w_tiles = []
                for kt in range(KT):
                    wt = w_pool.tile([P, W_N], F32R)
                    nc.sync.dma_start(
                        out=wt,
                        in_=w_d[kt * P:(kt + 1) * P, nsl].bitcast(F32R),
                    )
                    w_tiles.append(wt)
                b_t = b_pool.tile([RANK, W_N], F32R)
                nc.sync.dma_start(out=b_t, in_=b_d[:, nsl].bitcast(F32R))
                bias_sl = bias_d[nsl]
                bias_t = bias_pool.tile([P, W_N], F32)
                nc.sync.dma_start(
                    out=bias_t,
                    in_=bass.AP(tensor=bias_sl.tensor, offset=bias_sl.offset,
                                ap=[[0, P], *bias_sl.ap]),
                )

                psums = [psum_pool.tile([P, W_N], F32, tag="ps",
                                        name=f"psum_{nci}_{mt}")
                         for mt in range(MT)]
                for kt in range(KT):
                    for mt in range(MT):
                        nc.tensor.matmul(
                            psums[mt],
                            xT[:, kt, mt * P:(mt + 1) * P],
                            w_tiles[kt],
                            start=(kt == 0),
                            stop=False,
                        )
                for mt in range(MT):
                    nc.tensor.matmul(
                        psums[mt],
                        xat[:, mt * P:(mt + 1) * P],
                        b_t,
                        start=False,
                        stop=True,
                    )
                    ob = out_pool.tile([P, W_N], F32)
                    nc.vector.tensor_add(out=ob, in0=psums[mt], in1=bias_t)
                    nc.sync.dma_start(
                        out=out_d[mt * P:(mt + 1) * P, nsl], in_=ob)

    nc.compile()
    return nc


def get_nc():
    global _NC_CACHE
    if _NC_CACHE is None:
        _NC_CACHE = _build_nc()
    return _NC_CACHE


def make_in_maps(x, W, bias, lora_A, lora_B, scaling):
    x2 = np.ascontiguousarray(np.asarray(x, dtype=np.float32)).reshape(M_FULL, D_IN)
    w = np.ascontiguousarray(np.asarray(W, dtype=np.float32))
    b = np.ascontiguousarray(np.asarray(bias, dtype=np.float32))
    a = np.ascontiguousarray(np.asarray(lora_A, dtype=np.float32))
    s = np.float32(np.asarray(scaling).astype(np.float64))
    bs = np.ascontiguousarray(s * np.asarray(lora_B, dtype=np.float32))
    return [
        {
            "x": x2[c * M_C:(c + 1) * M_C],
            "w": w,
            "bias": b,
            "lora_a": a,
            "lora_b": bs,
        }
        for c in range(N_CORES)
    ]


def kernel(x, W, bias, lora_A, lora_B, scaling):
    nc = get_nc()
    in_maps = make_in_maps(x, W, bias, lora_A, lora_B, scaling)
    res = run_bass_kernel_spmd(nc, in_maps, core_ids=list(range(N_CORES)))
    out = np.concatenate([res.results[c]["out"] for c in range(N_CORES)], axis=0)
    return out.reshape(BATCH, SEQ, D_OUT)


# revision 5
# speedup vs baseline: 1.4207x; 1.4207x over previous
"""LoRA linear layer (out = x @ (W + s*A@B) + bias) on 8 Trainium2 NeuronCores.

Sharding: data-parallel over rows of x (M = 4*2048 = 8192 -> 1024 rows/core);
each core computes its row-slice against the full weight matrix. The x slice
is supplied in [K, M] layout (pure layout transform done while sharding) so
the contraction dim lands on SBUF partitions.

Per-core kernel (all fp32r = fp32 storage, ~fp22 multiply, fp32 accumulate):
  - stationary = W tile [128k x 128n], moving = xT [128k x 512m]; 32 K-tile
    matmuls accumulate each [128n x 512m] PSUM tile (out is computed
    transposed; the host transposes it back)
  - LoRA: xAT = A^T @ xT (rank 16) computed once on device; a 33rd rank-16
    matmul per PSUM tile adds (xA @ sB)^T into the same accumulation
  - bias is added during the PSUM -> SBUF copy on the scalar engine
    (per-partition bias = per-output-channel in the transposed layout)
"""
import numpy as np

import concourse.bass as bass
import concourse.tile as tile
from concourse import bacc, mybir
from concourse.bass_utils import run_bass_kernel_spmd

P = 128
N_CORES = 8
BATCH, SEQ = 4, 2048
D_IN, D_OUT, RANK = 4096, 4096, 16
M_FULL = BATCH * SEQ          # 8192
M_C = M_FULL // N_CORES       # 1024 rows per core
KT = D_IN // P                # 32 k-tiles
MC = M_C // 512               # 2 moving chunks of 512
NTP = D_OUT // 256            # 16 n-tile-pairs (W loaded 256 cols at a time)
F32 = mybir.dt.float32
F32R = mybir.dt.float32r

_NC_CACHE = None


def _build_nc():
    nc = bacc.Bacc("TRN2", target_bir_lowering=False, debug=False,
                   num_devices=N_CORES)
    xt_d = nc.dram_tensor("xt", [D_IN, M_C], F32, kind="ExternalInput").ap()
    w_d = nc.dram_tensor("w", [D_IN, D_OUT], F32, kind="ExternalInput").ap()
    bias_d = nc.dram_tensor("bias", [D_OUT], F32, kind="ExternalInput").ap()
    a_d = nc.dram_tensor("lora_a", [D_IN, RANK], F32, kind="ExternalInput").ap()
    b_d = nc.dram_tensor("lora_b", [RANK, D_OUT], F32, kind="ExternalInput").ap()
    outt_d = nc.dram_tensor("outt", [D_OUT, M_C], F32, kind="ExternalOutput").ap()

    with tile.TileContext(nc) as tc:
        with (
            tc.tile_pool(name="singles", bufs=1) as singles,
            tc.tile_pool(name="wts", bufs=40) as w_pool,
            tc.tile_pool(name="bt", bufs=3) as b_pool,
            tc.tile_pool(name="outs", bufs=4) as out_pool,
            tc.tile_pool(name="psum", bufs=8, space="PSUM") as psum_pool,
        ):
            # per-output-channel bias striped so channel lands on partition:
            # bias_cols[p, nt] = bias[nt*128 + p]
            bias_cols = singles.tile([P, D_OUT // P], F32)
            nc.sync.dma_start(
                out=bias_cols, in_=bias_d.rearrange("(nt p) -> p nt", p=P))

            # lora_A striped k-on-partitions: a_sb[p, kt, r] = A[kt*128+p, r]
            a_sb = singles.tile([P, KT, RANK], F32R)
            nc.sync.dma_start(
                out=a_sb,
                in_=a_d.bitcast(F32R).rearrange("(kt p) r -> p kt r", p=P),
            )

            # x slice in [k, m] layout, one DMA per k-tile
            xT = singles.tile([P, KT, M_C], F32R)
            for kt in range(KT):
                nc.sync.dma_start(
                    out=xT[:, kt, :],
                    in_=xt_d[kt * P:(kt + 1) * P, :].bitcast(F32R),
                )

            # xAT[r, m] = sum_k A[k, r] * xT[k, m]   (rank 16)
            xat = singles.tile([RANK, M_C], F32R)
            for mc in range(MC):
                xp = psum_pool.tile([RANK, 512], F32, tag="ps")
                for kt in range(KT):
                    nc.tensor.matmul(
                        xp,
                        a_sb[:, kt, :],
                        xT[:, kt, mc * 512:(mc + 1) * 512],
                        start=(kt == 0),
                        stop=(kt == KT - 1),
                    )
                nc.vector.tensor_copy(
                    out=xat[:, mc * 512:(mc + 1) * 512], in_=xp)

            # main loop: out^T[n, m] accumulated per [128n x 512m] PSUM tile
            for ntp in range(NTP):
                nsl = slice(ntp * 256, (ntp + 1) * 256)
                w_tiles = []
                for kt in range(KT):
                    wt = w_pool.tile([P, 256], F32R)
                    nc.sync.dma_start(
                        out=wt,
                        in_=w_d[kt * P:(kt + 1) * P, nsl].bitcast(F32R),
                    )
                    w_tiles.append(wt)
                bt = b_pool.tile([RANK, 256], F32R)
                nc.sync.dma_start(out=bt, in_=b_d[:, nsl].bitcast(F32R))

                for sub in range(2):
                    nt = ntp * 2 + sub
                    psums = [psum_pool.tile([P, 512], F32, tag="ps",
                                            name=f"ps_{nt}_{mc}")
                             for mc in range(MC)]
                    for kt in range(KT):
                        for mc in range(MC):
                            nc.tensor.matmul(
                                psums[mc],
                                w_tiles[kt][:, sub * P:(sub + 1) * P],
                                xT[:, kt, mc * 512:(mc + 1) * 512],
                                start=(kt == 0),
                                stop=False,
                            )
                    for mc in range(MC):
                        nc.tensor.matmul(
                            psums[mc],
                            bt[:, sub * P:(sub + 1) * P],
                            xat[:, mc * 512:(mc + 1) * 512],
                            start=False,
                            stop=True,
                        )
                        ob = out_pool.tile([P, 512], F32)
                        nc.scalar.activation(
                            ob, psums[mc],
                            mybir.ActivationFunctionType.Identity,
                            bias=bias_cols[:, nt:nt + 1],
                        )
                        nc.sync.dma_start(
                            out=outt_d[nt * P:(nt + 1) * P,
                                       mc * 512:(mc + 1) * 512],
                            in_=ob,
                        )

    nc.compile()
    return nc


def get_nc():
    global _NC_CACHE
    if _NC_CACHE is None:
        _NC_CACHE = _build_nc()
    return _NC_CACHE


def make_in_maps(x, W, bias, lora_A, lora_B, scaling):
    x2 = np.asarray(x, dtype=np.float32).reshape(M_FULL, D_IN)
    w = np.ascontiguousarray(np.asarray(W, dtype=np.float32))
    b = np.ascontiguousarray(np.asarray(bias, dtype=np.float32))
    a = np.ascontiguousarray(np.asarray(lora_A, dtype=np.float32))
    s = np.float32(np.asarray(scaling).astype(np.float64))
    bs = np.ascontiguousarray(s * np.asarray(lora_B, dtype=np.float32))
    return [
        {
            "xt": np.ascontiguousarray(x2[c * M_C:(c + 1) * M_C].T),
            "w": w,
            "bias": b,
            "lora_a": a,
            "lora_b": bs,
        }
        for c in range(N_CORES)
    ]


def assemble_output(results):
    """results: list of per-core dicts with 'outt' [D_OUT, M_C]."""
    out = np.concatenate(
        [results[c]["outt"].T for c in range(N_CORES)], axis=0)
    return np.ascontiguousarray(out).reshape(BATCH, SEQ, D_OUT)


def kernel(x, W, bias, lora_A, lora_B, scaling):
    nc = get_nc()
    in_maps = make_in_maps(x, W, bias, lora_A, lora_B, scaling)
    res = run_bass_kernel_spmd(nc, in_maps, core_ids=list(range(N_CORES)))
    return assemble_output(res.results)
